# revision 6
# baseline (speedup 1.0000x reference)
"""BiLSTM tagger on 8 TRN2 NeuronCores.

Strategy (hardcoded for B=64,T=512,V=30000,E=128,H=256,TAGS=50):
  - Data-parallel: batch sharded 8 ways (8 sequences/core); weights replicated.
  - Per core: embedding gather (indirect DMA) -> PE transpose -> x^T in SBUF;
    input projections xg = W_ih_aug @ [x; 1-m; 1] precomputed for all t as big
    matmuls into DRAM scratch; recurrences (l1 fwd+bwd interleaved, then l2
    fwd+bwd) as dynamic Tile loops, 16 steps per iteration; classifier matmul.
  - Masking: the (1-m) feature adds +/-60 to the f/i gate pre-activations at
    masked steps, freezing c exactly (sigmoid saturates to 1.0/0.0 in fp32).
    Backward-direction h is then exactly 0 at masked steps. Forward l2 output
    h is repaired with a per-step output-side blend using m / (1-m) planes
    carried in the xg stream. l1f's garbage masked outputs only feed masked
    l2 steps, where c is frozen anyway.
  - Gate layout: gates on partitions (8 chunks of 128 = [i0 i1 f0 f1 o0 o1 g0 g1]),
    batch on free dim -> cheap pointwise; Whh stationary [128h x 128gate] bf16
    tiles (FWL), h moving [128, 8].
"""
import sys

sys.path.insert(0, "/opt/trn_rl_repo")
import contextlib

import numpy as np
import ml_dtypes

import concourse.bass as bass
import concourse.bacc as bacc
import concourse.mybir as mybir
import concourse.tile as tile
from concourse.bass import ds
from concourse.bass_utils import run_bass_kernel_spmd
from concourse.masks import make_identity

B, T, V, E, H, TAGS = 64, 512, 30000, 128, 256, 50
NCORES = 8
Bc = B // NCORES          # 8 sequences per core
TB = T * Bc               # 4096 tokens per core
STEPS_PER_BODY = 16
NBODY = T // STEPS_PER_BODY  # 32

f32 = mybir.dt.float32
bf16 = mybir.dt.bfloat16
i32 = mybir.dt.int32

UNITS = ("1f", "1b", "2f", "2b")
KCNT = {"1f": 1, "1b": 1, "2f": 4, "2b": 4}       # 128-row K chunks of x features
MCNT = {"1f": 8, "1b": 8, "2f": 12, "2b": 8}      # 128-row output chunks
REV = {"1f": False, "1b": True, "2f": False, "2b": True}

# gate chunk order i0 i1 f0 f1 o0 o1 g0 g1 (torch row order is i f g o)
PERM = np.concatenate([np.arange(0, 256), np.arange(256, 512),
                       np.arange(768, 1024), np.arange(512, 768)])

_CACHE = {}


def _prep_unit_weights(Wih, Whh, bih, bhh, m_cnt):
    """Host-side weight marshalling for one LSTM direction."""
    din = Wih.shape[1]
    Wp = np.asarray(Wih)[PERM]          # [1024, din]
    Up = np.asarray(Whh)[PERM]          # [1024, 256]
    bp = (np.asarray(bih) + np.asarray(bhh))[PERM]  # [1024]
    M = m_cnt * 128
    k_cnt = din // 128
    # x-part lhsT: [din, M] -> k-chunk-major cols [128, k_cnt*M]
    WT = np.zeros((din, M), np.float32)
    WT[:, :1024] = Wp.T
    wx = np.concatenate([WT[k * 128:(k + 1) * 128, :] for k in range(k_cnt)],
                        axis=1).astype(ml_dtypes.bfloat16)  # [128, k_cnt*M]
    # aug lhsT rows: feature0 = (1-m), feature1 = 1
    wa = np.zeros((2, M), np.float32)
    wa[0, 0:256] = -60.0   # i rows: -60*(1-m)
    wa[0, 256:512] = 60.0  # f rows: +60*(1-m)
    wa[1, :1024] = bp
    if m_cnt == 12:        # l2f extra planes: m, m, 1-m, 1-m
        wa[0, 1024:1280] = -1.0
        wa[1, 1024:1280] = 1.0
        wa[0, 1280:1536] = 1.0
    wa = wa.astype(ml_dtypes.bfloat16)
    # Whh lhsT: [256, 1024] -> [128, 2*1024]
    UT = Up.T.astype(np.float32)
    wh = np.concatenate([UT[0:128, :], UT[128:256, :]], axis=1).astype(ml_dtypes.bfloat16)
    return wx, wa, wh


def _build_program(stage="full", repeat=1):
    nc = bacc.Bacc("TRN2", target_bir_lowering=False, debug=False, num_devices=NCORES)
    emb_d = nc.dram_tensor("emb", [V, E], f32, kind="ExternalInput")
    words_d = nc.dram_tensor("words", [TB, 1], i32, kind="ExternalInput")
    aug_d = nc.dram_tensor("aug", [2, TB], bf16, kind="ExternalInput")
    wxd, wad, whd, xgd = {}, {}, {}, {}
    for u in UNITS:
        wxd[u] = nc.dram_tensor(f"w{u}x", [128, KCNT[u] * MCNT[u] * 128], bf16, kind="ExternalInput")
        wad[u] = nc.dram_tensor(f"w{u}a", [2, MCNT[u] * 128], bf16, kind="ExternalInput")
        whd[u] = nc.dram_tensor(f"w{u}h", [128, 2048], bf16, kind="ExternalInput")
        xgd[u] = nc.dram_tensor(f"xg{u}", [MCNT[u], 128, TB], f32)
    clsx_d = nc.dram_tensor("clsx", [128, 4 * TAGS], bf16, kind="ExternalInput")
    clsb_d = nc.dram_tensor("clsb", [TAGS, 1], f32, kind="ExternalInput")
    logits_d = nc.dram_tensor("logits", [TAGS, TB], f32, kind="ExternalOutput")

    ctx = contextlib.ExitStack()
    with tile.TileContext(nc) as tc, ctx:
        pp = ctx.enter_context(tc.tile_pool(name="persist", bufs=1))
        xT = pp.tile([128, TB], bf16, tag="xT")
        aug_sb = pp.tile([2, TB], bf16, tag="aug")
        ident = pp.tile([128, 128], f32, tag="ident")
        wx_sb = {u: pp.tile([128, KCNT[u] * MCNT[u] * 128], bf16, tag=f"wx{u}", name=f"wx{u}") for u in UNITS}
        wa_sb = {u: pp.tile([2, MCNT[u] * 128], bf16, tag=f"wa{u}", name=f"wa{u}") for u in UNITS}
        wh_sb = {u: pp.tile([128, 2048], bf16, tag=f"wh{u}", name=f"wh{u}") for u in UNITS}
        cls_sb = pp.tile([128, 4 * TAGS], bf16, tag="clsx")
        clsb_sb = pp.tile([TAGS, 1], f32, tag="clsb")
        hs = {u: pp.tile([128, T, 2, Bc], bf16, tag=f"hs{u}", name=f"hs{u}") for u in UNITS}
        o2f_sb = pp.tile([128, T, 2, Bc], bf16, tag="o2f")
        hcar = {u: pp.tile([128, 2, Bc], bf16, tag=f"hc{u}", name=f"hc{u}") for u in UNITS}
        ccar = {u: pp.tile([128, 2, Bc], f32, tag=f"cc{u}", name=f"cc{u}") for u in UNITS}
        o2f_car = pp.tile([128, 2, Bc], bf16, tag="o2fc")

        # ---- load weights / constants
        make_identity(nc, ident[:])
        for u in UNITS:
            nc.sync.dma_start(wx_sb[u][:], wxd[u][:])
            nc.sync.dma_start(wa_sb[u][:], wad[u][:])
            nc.sync.dma_start(wh_sb[u][:], whd[u][:])
        nc.sync.dma_start(cls_sb[:], clsx_d[:])
        nc.sync.dma_start(clsb_sb[:], clsb_d[:])
        nc.sync.dma_start(aug_sb[:], aug_d[:])
        for u in UNITS:
            nc.vector.memset(hcar[u][:, :, :], 0.0)
            nc.vector.memset(ccar[u][:, :, :], 0.0)
        nc.vector.memset(o2f_car[:, :, :], 0.0)

        # ---- embedding gather + transpose into xT
        for _rep in range(repeat):
         with nc.named_scope("gather"), \
             tc.tile_pool(name=f"gat{_rep}", bufs=3) as gp, \
             tc.tile_pool(name=f"gps{_rep}", bufs=3, space="PSUM") as gps:
            for n in range(TB // 128):
                idx = gp.tile([128, 1], i32, tag="idx")
                nc.sync.dma_start(idx[:], words_d[n * 128:(n + 1) * 128, :])
                xt = gp.tile([128, 128], f32, tag="xt")
                nc.gpsimd.indirect_dma_start(
                    out=xt[:], out_offset=None, in_=emb_d[:, :],
                    in_offset=bass.IndirectOffsetOnAxis(ap=idx[:, :1], axis=0))
                pst = gps.tile([128, 128], f32, tag="pst")
                nc.tensor.transpose(out=pst[:], in_=xt[:], identity=ident[:])
                nc.vector.tensor_copy(xT[:, n * 128:(n + 1) * 128], pst[:])

        # ---- xg precompute
        def xg_precompute(u, rhs_of_k, _rep=0):
            m_cnt, k_cnt = MCNT[u], KCNT[u]
            with nc.named_scope(f"xg{u}"), \
                 tc.tile_pool(name=f"xp{u}{_rep}", bufs=4, space="PSUM") as xps, \
                 tc.tile_pool(name=f"xs{u}{_rep}", bufs=4) as xsb:
                for n in range(TB // 512):
                    nsl = slice(n * 512, (n + 1) * 512)
                    for m in range(m_cnt):
                        psm = xps.tile([128, 512], f32, tag="ps")
                        first = True
                        if m < 8:  # gate chunks get the x contribution
                            for k in range(k_cnt):
                                nc.tensor.matmul(
                                    out=psm[:],
                                    lhsT=wx_sb[u][:, (k * m_cnt + m) * 128:(k * m_cnt + m + 1) * 128],
                                    rhs=rhs_of_k(k, n),
                                    start=first, stop=False)
                                first = False
                        nc.tensor.matmul(
                            out=psm[:],
                            lhsT=wa_sb[u][:, m * 128:(m + 1) * 128],
                            rhs=aug_sb[:, nsl],
                            start=first, stop=True)
                        stg = xsb.tile([128, 512], f32, tag="stg")
                        if (n + m) % 2 == 0:
                            nc.vector.tensor_copy(stg[:], psm[:])
                        else:
                            nc.scalar.activation(stg[:], psm[:],
                                                 mybir.ActivationFunctionType.Copy)
                        nc.sync.dma_start(xgd[u][m, :, nsl], stg[:])

        def l1_rhs(k, n):
            return xT[:, n * 512:(n + 1) * 512]

        if stage != "gather":
            for _rep in range(repeat):
                xg_precompute("1f", l1_rhs, _rep)
                xg_precompute("1b", l1_rhs, _rep)

        # ---- recurrence phase
        def phase(units, _rep=0):
            with nc.named_scope(f"ph{units[0]}"), \
                 tc.tile_pool(name=f"rc{units[0]}{_rep}", bufs=2) as rp, \
                 tc.tile_pool(name=f"rps{units[0]}{_rep}", bufs=4, space="PSUM") as rps, \
                 tc.tile_pool(name=f"rtmp{units[0]}{_rep}", bufs=3) as tp:
                with tc.For_i(0, NBODY, hint_engines=(mybir.EngineType.PE,)) as i:
                    for u in units:
                        m_cnt = MCNT[u]
                        rev = REV[u]
                        xb = rp.tile([128, m_cnt, 128], f32, tag=f"xb{u}")
                        if rev:
                            col0 = i * (-128) + (TB - 128)
                            t0 = i * (-STEPS_PER_BODY) + (T - STEPS_PER_BODY)
                        else:
                            col0 = i * 128
                            t0 = i * STEPS_PER_BODY
                        for m in range(m_cnt):
                            nc.sync.dma_start(xb[:, m, :], xgd[u][m, :, ds(col0, 128)])
                        hstage = rp.tile([128, STEPS_PER_BODY, 2, Bc], bf16, tag=f"hst{u}")
                        if u == "2f":
                            ostage = rp.tile([128, STEPS_PER_BODY, 2, Bc], bf16, tag="ost")
                        for us in range(STEPS_PER_BODY):
                            slot = (STEPS_PER_BODY - 1 - us) if rev else us
                            bc = slot * Bc
                            if us == 0:
                                hprev = hcar[u]
                            else:
                                pslot = slot + 1 if rev else slot - 1
                                hprev = hstage[:, pslot, :, :]
                            psm = rps.tile([128, 8, Bc], f32, tag="g")
                            for m in range(8):
                                for k in range(2):
                                    nc.tensor.matmul(
                                        out=psm[:, m, :],
                                        lhsT=wh_sb[u][:, (k * 8 + m) * 128:(k * 8 + m + 1) * 128],
                                        rhs=hprev[:, k, :] if us else hprev[:, k, :],
                                        start=(k == 0), stop=(k == 1))
                            g = tp.tile([128, 8, Bc], f32, tag="gs")
                            nc.vector.tensor_tensor(
                                out=g[:, :, :], in0=psm[:, :, :],
                                in1=xb[:, 0:8, bc:bc + Bc], op=mybir.AluOpType.add)
                            sg = tp.tile([128, 8, Bc], f32, tag="sg")
                            nc.scalar.activation(sg[:, 0:6, :], g[:, 0:6, :],
                                                 mybir.ActivationFunctionType.Sigmoid)
                            nc.scalar.activation(sg[:, 6:8, :], g[:, 6:8, :],
                                                 mybir.ActivationFunctionType.Tanh)
                            t1 = tp.tile([128, 2, Bc], f32, tag="t1")
                            nc.vector.tensor_tensor(out=t1[:, :, :], in0=sg[:, 0:2, :],
                                                    in1=sg[:, 6:8, :], op=mybir.AluOpType.mult)
                            csf = tp.tile([128, 2, Bc], f32, tag="csf")
                            nc.vector.tensor_tensor(out=csf[:, :, :], in0=sg[:, 2:4, :],
                                                    in1=ccar[u][:, :, :], op=mybir.AluOpType.mult)
                            nc.vector.tensor_tensor(out=ccar[u][:, :, :], in0=csf[:, :, :],
                                                    in1=t1[:, :, :], op=mybir.AluOpType.add)
                            tc2 = tp.tile([128, 2, Bc], f32, tag="tc2")
                            nc.scalar.activation(tc2[:, :, :], ccar[u][:, :, :],
                                                 mybir.ActivationFunctionType.Tanh)
                            nc.vector.tensor_tensor(out=hstage[:, slot, :, :], in0=sg[:, 4:6, :],
                                                    in1=tc2[:, :, :], op=mybir.AluOpType.mult)
                            if u == "2f":
                                ma = tp.tile([128, 2, Bc], f32, tag="ma")
                                nc.vector.tensor_tensor(out=ma[:, :, :], in0=hstage[:, slot, :, :],
                                                        in1=xb[:, 8:10, bc:bc + Bc],
                                                        op=mybir.AluOpType.mult)
                                oprev = o2f_car if us == 0 else ostage[:, slot - 1, :, :]
                                mb = tp.tile([128, 2, Bc], f32, tag="mb")
                                nc.vector.tensor_tensor(out=mb[:, :, :], in0=oprev[:, :, :],
                                                        in1=xb[:, 10:12, bc:bc + Bc],
                                                        op=mybir.AluOpType.mult)
                                nc.vector.tensor_tensor(out=ostage[:, slot, :, :], in0=ma[:, :, :],
                                                        in1=mb[:, :, :], op=mybir.AluOpType.add)
                        # flush staged h history
                        nc.vector.tensor_copy(hs[u][:, ds(t0, STEPS_PER_BODY), :, :],
                                              hstage[:, :, :, :])
                        last_slot = 0 if rev else STEPS_PER_BODY - 1
                        nc.vector.tensor_copy(hcar[u][:, :, :], hstage[:, last_slot, :, :])
                        if u == "2f":
                            nc.vector.tensor_copy(o2f_sb[:, ds(t0, STEPS_PER_BODY), :, :],
                                                  ostage[:, :, :, :])
                            nc.vector.tensor_copy(o2f_car[:, :, :],
                                                  ostage[:, STEPS_PER_BODY - 1, :, :])

        if stage in ("ph1", "xg2", "full"):
            for _rep in range(repeat):
                phase(("1f", "1b"), _rep)

        def l2_rhs(k, n):
            src = hs["1f"] if k < 2 else hs["1b"]
            return src[:, n * 64:(n + 1) * 64, k % 2, :]

        if stage in ("xg2", "full"):
            for _rep in range(repeat):
                xg_precompute("2f", l2_rhs, _rep)
                xg_precompute("2b", l2_rhs, _rep)

        if stage == "full":
            for _rep in range(repeat):
                phase(("2f", "2b"), _rep)

        # ---- classifier
        if stage != "full":
            with tc.tile_pool(name="dum", bufs=1) as dp:
                dmy = dp.tile([TAGS, 512], f32, tag="dmy")
                nc.vector.memset(dmy[:], 0.0)
                nc.sync.dma_start(logits_d[:, 0:512], dmy[:])
        else:
         with nc.named_scope("cls"), \
             tc.tile_pool(name="cl", bufs=3) as cp, \
             tc.tile_pool(name="cps", bufs=3, space="PSUM") as cps:
            for n in range(TB // 512):
                psm = cps.tile([TAGS, 512], f32, tag="ps")
                for k in range(4):
                    src = o2f_sb if k < 2 else hs["2b"]
                    nc.tensor.matmul(
                        out=psm[:],
                        lhsT=cls_sb[:, k * TAGS:(k + 1) * TAGS],
                        rhs=src[:, n * 64:(n + 1) * 64, k % 2, :],
                        start=(k == 0), stop=(k == 3))
                lg = cp.tile([TAGS, 512], f32, tag="lg")
                nc.vector.tensor_scalar_add(lg[:], psm[:], clsb_sb[:, :1])
                nc.sync.dma_start(logits_d[:, n * 512:(n + 1) * 512], lg[:])

    nc.compile()
    return nc


def _make_in_maps(inputs):
    words = np.asarray(inputs["words"]).astype(np.int32)
    lengths = np.asarray(inputs["lengths"]).astype(np.int32)
    emb = np.asarray(inputs["emb"], dtype=np.float32)
    mask = (lengths[:, None] > np.arange(T)[None, :]).astype(np.float32)
    wprep = {u: _prep_unit_weights(inputs[f"l{u}_Wih"], inputs[f"l{u}_Whh"],
                                   inputs[f"l{u}_bih"], inputs[f"l{u}_bhh"], MCNT[u])
             for u in UNITS}
    clsW = np.asarray(inputs["cls_W"], dtype=np.float32)
    CT = clsW.T
    clsx = np.concatenate([CT[k * 128:(k + 1) * 128, :] for k in range(4)],
                          axis=1).astype(ml_dtypes.bfloat16)
    clsb = np.asarray(inputs["cls_b"], dtype=np.float32).reshape(TAGS, 1)
    in_maps = []
    for c in range(NCORES):
        bsl = slice(c * Bc, (c + 1) * Bc)
        w_c = words[bsl]
        m_c = mask[bsl]
        words_tm = np.ascontiguousarray(w_c.T).reshape(TB, 1)
        aug = np.stack([(1.0 - m_c.T).reshape(TB), np.ones(TB, np.float32)]
                       ).astype(ml_dtypes.bfloat16)
        im = {"emb": emb, "words": words_tm, "aug": aug,
              "clsx": clsx, "clsb": clsb}
        for u in UNITS:
            wx, wa, wh = wprep[u]
            im[f"w{u}x"] = wx
            im[f"w{u}a"] = wa
            im[f"w{u}h"] = wh
        in_maps.append(im)
    return in_maps


def kernel(**inputs):
    words = np.asarray(inputs["words"]).astype(np.int32)      # [B, T]
    lengths = np.asarray(inputs["lengths"]).astype(np.int32)  # [B]
    emb = np.asarray(inputs["emb"], dtype=np.float32)

    if "nc" not in _CACHE:
        _CACHE["nc"] = _build_program()
    nc = _CACHE["nc"]

    mask = (lengths[:, None] > np.arange(T)[None, :]).astype(np.float32)  # [B,T]
    wprep = {u: _prep_unit_weights(inputs[f"l{u}_Wih"], inputs[f"l{u}_Whh"],
                                   inputs[f"l{u}_bih"], inputs[f"l{u}_bhh"], MCNT[u])
             for u in UNITS}
    clsW = np.asarray(inputs["cls_W"], dtype=np.float32)      # [50, 512]
    CT = clsW.T  # [512, 50]
    clsx = np.concatenate([CT[k * 128:(k + 1) * 128, :] for k in range(4)],
                          axis=1).astype(ml_dtypes.bfloat16)  # [128, 200]
    clsb = np.asarray(inputs["cls_b"], dtype=np.float32).reshape(TAGS, 1)

    in_maps = []
    for c in range(NCORES):
        bsl = slice(c * Bc, (c + 1) * Bc)
        w_c = words[bsl]                      # [Bc, T]
        m_c = mask[bsl]                       # [Bc, T]
        words_tm = np.ascontiguousarray(w_c.T).reshape(TB, 1)
        aug = np.stack([(1.0 - m_c.T).reshape(TB), np.ones(TB, np.float32)]
                       ).astype(ml_dtypes.bfloat16)           # [2, TB]
        im = {"emb": emb, "words": words_tm, "aug": aug,
              "clsx": clsx, "clsb": clsb}
        for u in UNITS:
            wx, wa, wh = wprep[u]
            im[f"w{u}x"] = wx
            im[f"w{u}a"] = wa
            im[f"w{u}h"] = wh
        in_maps.append(im)

    _CACHE["in_maps"] = in_maps
    res = run_bass_kernel_spmd(nc, in_maps, list(range(NCORES)))
    out = np.empty((B, T, TAGS), np.float32)
    for c in range(NCORES):
        lg = res.results[c]["logits"]          # [50, TB], col = t*Bc + b
        out[c * Bc:(c + 1) * Bc] = lg.reshape(TAGS, T, Bc).transpose(2, 1, 0)
    return out


def bench(inputs):
    """Run once with NTFF tracing; returns HW exec_time_ns (and stashes trace)."""
    kernel(**inputs)  # ensure program built/cached
    nc = _CACHE["nc"]
    in_maps = _CACHE["in_maps"]
    import tempfile
    tmpdir = tempfile.mkdtemp(prefix="bilstm_trace_")
    res = run_bass_kernel_spmd(nc, in_maps, list(range(NCORES)), trace=True,
                               tmpdir=tmpdir)
    _CACHE["trace_dir"] = tmpdir
    _CACHE["last_bench"] = res
    print("trace dir:", tmpdir)
    if res.per_core_scope_times:
        for scope, times in res.per_core_scope_times.items():
            print(f"scope {scope}: {times}")
    return res.exec_time_ns


if __name__ == "__main__":
    import reference
    inputs = {k: np.asarray(v) for k, v in reference.setup_inputs().items()}
    got = kernel(**inputs)
    print(got.shape, got.dtype)



# revision 10
# speedup vs baseline: 1.6402x; 1.6402x over previous
"""BiLSTM tagger on 8 TRN2 NeuronCores.

Strategy (hardcoded for B=64,T=512,V=30000,E=128,H=256,TAGS=50):
  - Data-parallel: batch sharded 8 ways (8 sequences/core); weights replicated.
  - Per core: embedding gather (indirect DMA) -> PE transpose -> x^T in SBUF;
    input projections xg = W_ih_aug @ [x; 1-m; 1] precomputed for all t as big
    matmuls into DRAM scratch; recurrences (l1 fwd+bwd step-interleaved, then
    l2 fwd+bwd) as dynamic Tile loops, 16 steps per iteration; classifier.
  - Masking: the (1-m) feature adds +/-60 to the f/i gate pre-activations at
    masked steps, freezing c exactly (sigmoid saturates to 1.0/0.0 in fp32).
    Backward-direction h is then exactly 0 at masked steps. Forward l2 output
    h is repaired with copy_predicated (hold previous output where mask=0).
    l1f's garbage masked outputs only feed masked l2 steps, where c is frozen.
  - Gate layout: gates on partitions, chunk order [i0 i1 f0 f1 g0 g1 o0 o1],
    batch on free dim. g rows pre-scaled by 2 so tanh(z)=2*sigmoid(2z)-1 lets
    one sigmoid cover i,f,g; per-step xg add done ON THE PE via an fp32
    identity matmul accumulating into PSUM (h-independent, issues early).
    PSUM split per step into main(i,f,g: 6 chunks) and o(2 chunks) banks so
    the main sigmoid doesn't wait for the o-gate matmuls.
"""
import sys

sys.path.insert(0, "/opt/trn_rl_repo")
import contextlib

import numpy as np
import ml_dtypes

import concourse.bass as bass
import concourse.bacc as bacc
import concourse.mybir as mybir
import concourse.tile as tile
from concourse.bass import ds
from concourse.bass_utils import run_bass_kernel_spmd
from concourse.masks import make_identity

B, T, V, E, H, TAGS = 64, 512, 30000, 128, 256, 50
NCORES = 8
Bc = B // NCORES          # 8 sequences per core
TB = T * Bc               # 4096 tokens per core
STEPS_PER_BODY = 16
NBODY = T // STEPS_PER_BODY  # 32

f32 = mybir.dt.float32
bf16 = mybir.dt.bfloat16
i32 = mybir.dt.int32

UNITS = ("1f", "1b", "2f", "2b")
KCNT = {"1f": 1, "1b": 1, "2f": 4, "2b": 4}       # 128-row K chunks of x features
MCNT = {"1f": 8, "1b": 8, "2f": 10, "2b": 8}      # 128-row output chunks
REV = {"1f": False, "1b": True, "2f": False, "2b": True}

_CACHE = {}


def _prep_unit_weights(Wih, Whh, bih, bhh, m_cnt):
    """Host-side weight marshalling for one LSTM direction.

    Row order already [i f g o]; g rows (512:768) scaled by 2 so that
    tanh(z) = 2*sigmoid(2z) - 1 can use the sigmoid table."""
    din = Wih.shape[1]
    Wp = np.asarray(Wih, np.float32).copy()          # [1024, din]
    Up = np.asarray(Whh, np.float32).copy()          # [1024, 256]
    bp = (np.asarray(bih) + np.asarray(bhh)).astype(np.float32)
    Wp[512:768] *= 2.0
    Up[512:768] *= 2.0
    bp = bp.copy()
    bp[512:768] *= 2.0
    M = m_cnt * 128
    k_cnt = din // 128
    # x-part lhsT: [din, M] -> k-chunk-major cols [128, k_cnt*M]
    WT = np.zeros((din, M), np.float32)
    WT[:, :1024] = Wp.T
    wx = np.concatenate([WT[k * 128:(k + 1) * 128, :] for k in range(k_cnt)],
                        axis=1).astype(ml_dtypes.bfloat16)  # [128, k_cnt*M]
    # aug lhsT rows: feature0 = (1-m), feature1 = 1
    wa = np.zeros((2, M), np.float32)
    wa[0, 0:256] = -60.0   # i rows: -60*(1-m)
    wa[0, 256:512] = 60.0  # f rows: +60*(1-m)
    wa[1, :1024] = bp
    if m_cnt == 10:        # l2f extra planes: m, m (for copy_predicated mask)
        wa[0, 1024:1280] = -1.0
        wa[1, 1024:1280] = 1.0
    wa = wa.astype(ml_dtypes.bfloat16)
    # Whh lhsT: [256, 1024] -> [128, 2*1024]
    UT = Up.T
    wh = np.concatenate([UT[0:128, :], UT[128:256, :]], axis=1).astype(ml_dtypes.bfloat16)
    return wx, wa, wh


def _build_program(stage="full", repeat=1):
    nc = bacc.Bacc("TRN2", target_bir_lowering=False, debug=False, num_devices=NCORES)
    emb_d = nc.dram_tensor("emb", [V, E], f32, kind="ExternalInput")
    words_d = nc.dram_tensor("words", [TB, 1], i32, kind="ExternalInput")
    aug_d = nc.dram_tensor("aug", [2, TB], bf16, kind="ExternalInput")
    wxd, wad, whd, xgd = {}, {}, {}, {}
    for u in UNITS:
        wxd[u] = nc.dram_tensor(f"w{u}x", [128, KCNT[u] * MCNT[u] * 128], bf16, kind="ExternalInput")
        wad[u] = nc.dram_tensor(f"w{u}a", [2, MCNT[u] * 128], bf16, kind="ExternalInput")
        whd[u] = nc.dram_tensor(f"w{u}h", [128, 2048], bf16, kind="ExternalInput")
        xgd[u] = nc.dram_tensor(f"xg{u}", [128, MCNT[u], T, Bc], f32)
    clsx_d = nc.dram_tensor("clsx", [128, 4 * TAGS], bf16, kind="ExternalInput")
    clsb_d = nc.dram_tensor("clsb", [TAGS, 1], f32, kind="ExternalInput")
    logits_d = nc.dram_tensor("logits", [TAGS, TB], f32, kind="ExternalOutput")

    ctx = contextlib.ExitStack()
    with tile.TileContext(nc) as tc, ctx:
        pp = ctx.enter_context(tc.tile_pool(name="persist", bufs=1))
        xT = pp.tile([128, TB], bf16, tag="xT")
        aug_sb = pp.tile([2, TB], bf16, tag="aug")
        ident = pp.tile([128, 128], f32, tag="ident")
        wx_sb = {u: pp.tile([128, KCNT[u] * MCNT[u] * 128], bf16, tag=f"wx{u}", name=f"wx{u}") for u in UNITS}
        wa_sb = {u: pp.tile([2, MCNT[u] * 128], bf16, tag=f"wa{u}", name=f"wa{u}") for u in UNITS}
        wh_sb = {u: pp.tile([128, 2048], bf16, tag=f"wh{u}", name=f"wh{u}") for u in UNITS}
        cls_sb = pp.tile([128, 4 * TAGS], bf16, tag="clsx")
        clsb_sb = pp.tile([TAGS, 1], f32, tag="clsb")
        hs = {u: pp.tile([128, T, 2, Bc], bf16, tag=f"hs{u}", name=f"hs{u}")
              for u in ("1f", "1b", "2b")}
        o2f_sb = pp.tile([128, T, 2, Bc], bf16, tag="o2f")
        hcar = {u: pp.tile([128, 2, Bc], bf16, tag=f"hc{u}", name=f"hc{u}") for u in UNITS}
        ccar = {u: pp.tile([128, 2, Bc], f32, tag=f"cc{u}", name=f"cc{u}") for u in UNITS}
        o2f_car = pp.tile([128, 2, Bc], bf16, tag="o2fc")

        # ---- load weights / constants
        make_identity(nc, ident[:])
        for u in UNITS:
            nc.sync.dma_start(wx_sb[u][:], wxd[u][:])
            nc.sync.dma_start(wa_sb[u][:], wad[u][:])
            nc.sync.dma_start(wh_sb[u][:], whd[u][:])
        nc.sync.dma_start(cls_sb[:], clsx_d[:])
        nc.sync.dma_start(clsb_sb[:], clsb_d[:])
        nc.sync.dma_start(aug_sb[:], aug_d[:])
        for u in UNITS:
            nc.vector.memset(hcar[u][:, :, :], 0.0)
            nc.vector.memset(ccar[u][:, :, :], 0.0)
        nc.vector.memset(o2f_car[:, :, :], 0.0)

        # ---- embedding gather + transpose into xT
        for _rep in range(repeat):
         with nc.named_scope("gather"), \
             tc.tile_pool(name=f"gat{_rep}", bufs=3) as gp, \
             tc.tile_pool(name=f"gps{_rep}", bufs=3, space="PSUM") as gps:
            for n in range(TB // 128):
                idx = gp.tile([128, 1], i32, tag="idx")
                nc.sync.dma_start(idx[:], words_d[n * 128:(n + 1) * 128, :])
                xt = gp.tile([128, 128], f32, tag="xt")
                nc.gpsimd.indirect_dma_start(
                    out=xt[:], out_offset=None, in_=emb_d[:, :],
                    in_offset=bass.IndirectOffsetOnAxis(ap=idx[:, :1], axis=0))
                pst = gps.tile([128, 128], f32, tag="pst")
                nc.tensor.transpose(out=pst[:], in_=xt[:], identity=ident[:])
                nc.vector.tensor_copy(xT[:, n * 128:(n + 1) * 128], pst[:])

        # ---- xg precompute into DRAM, layout [128, m, T, Bc]
        def xg_precompute(u, rhs_of_k, _rep=0):
            m_cnt, k_cnt = MCNT[u], KCNT[u]
            with nc.named_scope(f"xg{u}"), \
                 tc.tile_pool(name=f"xp{u}{_rep}", bufs=4, space="PSUM") as xps, \
                 tc.tile_pool(name=f"xs{u}{_rep}", bufs=4) as xsb:
                for n in range(TB // 512):
                    nsl = slice(n * 512, (n + 1) * 512)
                    for m in range(m_cnt):
                        psm = xps.tile([128, 512], f32, tag="ps")
                        first = True
                        if m < 8:  # gate chunks get the x contribution
                            for k in range(k_cnt):
                                nc.tensor.matmul(
                                    out=psm[:],
                                    lhsT=wx_sb[u][:, (k * m_cnt + m) * 128:(k * m_cnt + m + 1) * 128],
                                    rhs=rhs_of_k(k, n),
                                    start=first, stop=False)
                                first = False
                        nc.tensor.matmul(
                            out=psm[:],
                            lhsT=wa_sb[u][:, m * 128:(m + 1) * 128],
                            rhs=aug_sb[:, nsl],
                            start=first, stop=True)
                        stg = xsb.tile([128, 512], f32, tag="stg")
                        if (n + m) % 2 == 0:
                            nc.vector.tensor_copy(stg[:], psm[:])
                        else:
                            nc.scalar.activation(stg[:], psm[:],
                                                 mybir.ActivationFunctionType.Copy)
                        nc.sync.dma_start(
                            xgd[u][:, m, n * 64:(n + 1) * 64, :],
                            stg[:, :].rearrange("p (t b) -> p t b", b=Bc))

        def l1_rhs(k, n):
            return xT[:, n * 512:(n + 1) * 512]

        if stage != "gather":
            for _rep in range(repeat):
                xg_precompute("1f", l1_rhs, _rep)
                xg_precompute("1b", l1_rhs, _rep)

        # ---- recurrence phase: the two direction units step-interleaved
        def phase(units, _rep=0):
            with nc.named_scope(f"ph{units[0]}"), \
                 tc.tile_pool(name=f"rc{units[0]}{_rep}", bufs=2) as rp, \
                 tc.tile_pool(name=f"rpsA{units[0]}{_rep}", bufs=2, space="PSUM") as rpsA, \
                 tc.tile_pool(name=f"rpsB{units[0]}{_rep}", bufs=2, space="PSUM") as rpsB, \
                 tc.tile_pool(name=f"rtmp{units[0]}{_rep}", bufs=3) as tp:
                with tc.For_i(0, NBODY, hint_engines=(mybir.EngineType.PE,)) as i:
                    xb, hstage, t0s = {}, {}, {}
                    ostage = None
                    for u in units:
                        m_cnt = MCNT[u]
                        if REV[u]:
                            t0 = i * (-STEPS_PER_BODY) + (T - STEPS_PER_BODY)
                        else:
                            t0 = i * STEPS_PER_BODY
                        t0s[u] = t0
                        xb[u] = rp.tile([128, m_cnt, STEPS_PER_BODY, Bc], f32,
                                        tag=f"xb{u}", name=f"xb{u}")
                        nc.sync.dma_start(xb[u][:, :, :, :],
                                          xgd[u][:, :, ds(t0, STEPS_PER_BODY), :])
                        hstage[u] = rp.tile([128, STEPS_PER_BODY, 2, Bc], bf16,
                                            tag=f"hst{u}", name=f"hst{u}")
                        if u == "2f":
                            ostage = rp.tile([128, STEPS_PER_BODY, 2, Bc], bf16,
                                             tag="ost")
                    for us in range(STEPS_PER_BODY):
                        for u in units:
                            rev = REV[u]
                            slot = (STEPS_PER_BODY - 1 - us) if rev else us
                            if us == 0:
                                hprev = hcar[u]
                            else:
                                pslot = slot + 1 if rev else slot - 1
                                hprev = hstage[u][:, pslot, :, :]
                            # PSUM: main = [i0 i1 f0 f1 g0 g1], o = [o0 o1]
                            psm = rpsA.tile([128, 6, Bc], f32, tag=f"gm{u}")
                            pso = rpsB.tile([128, 2, Bc], f32, tag=f"go{u}")
                            # xg add on PE (h-independent, issues early)
                            nc.tensor.matmul(out=psm[:, :, :], lhsT=ident[:],
                                             rhs=xb[u][:, 0:6, slot, :],
                                             start=True, stop=False)
                            nc.tensor.matmul(out=pso[:, :, :], lhsT=ident[:],
                                             rhs=xb[u][:, 6:8, slot, :],
                                             start=True, stop=False)
                            for m in range(6):
                                for k in range(2):
                                    nc.tensor.matmul(
                                        out=psm[:, m, :],
                                        lhsT=wh_sb[u][:, (k * 8 + m) * 128:(k * 8 + m + 1) * 128],
                                        rhs=hprev[:, k, :],
                                        start=False, stop=(m == 5 and k == 1))
                            for m in range(6, 8):
                                for k in range(2):
                                    nc.tensor.matmul(
                                        out=pso[:, m - 6, :],
                                        lhsT=wh_sb[u][:, (k * 8 + m) * 128:(k * 8 + m + 1) * 128],
                                        rhs=hprev[:, k, :],
                                        start=False, stop=(m == 7 and k == 1))
                            sg = tp.tile([128, 8, Bc], f32, tag=f"sg{u}")
                            nc.scalar.activation(sg[:, 0:6, :], psm[:, :, :],
                                                 mybir.ActivationFunctionType.Sigmoid)
                            nc.scalar.activation(sg[:, 6:8, :], pso[:, :, :],
                                                 mybir.ActivationFunctionType.Sigmoid)
                            csf = tp.tile([128, 2, Bc], f32, tag=f"csf{u}")
                            nc.vector.tensor_tensor(out=csf[:, :, :], in0=sg[:, 2:4, :],
                                                    in1=ccar[u][:, :, :], op=mybir.AluOpType.mult)
                            gt = tp.tile([128, 2, Bc], f32, tag=f"gt{u}")
                            nc.vector.tensor_scalar(out=gt[:, :, :], in0=sg[:, 4:6, :],
                                                    scalar1=2.0, scalar2=-1.0,
                                                    op0=mybir.AluOpType.mult,
                                                    op1=mybir.AluOpType.add)
                            t1 = tp.tile([128, 2, Bc], f32, tag=f"t1{u}")
                            nc.vector.tensor_tensor(out=t1[:, :, :], in0=sg[:, 0:2, :],
                                                    in1=gt[:, :, :], op=mybir.AluOpType.mult)
                            nc.vector.tensor_tensor(out=ccar[u][:, :, :], in0=csf[:, :, :],
                                                    in1=t1[:, :, :], op=mybir.AluOpType.add)
                            tc2 = tp.tile([128, 2, Bc], f32, tag=f"tc2{u}")
                            nc.scalar.activation(tc2[:, :, :], ccar[u][:, :, :],
                                                 mybir.ActivationFunctionType.Tanh)
                            nc.vector.tensor_tensor(out=hstage[u][:, slot, :, :],
                                                    in0=sg[:, 6:8, :],
                                                    in1=tc2[:, :, :], op=mybir.AluOpType.mult)
                            if u == "2f":
                                oprev = o2f_car if us == 0 else ostage[:, slot - 1, :, :]
                                nc.vector.tensor_copy(ostage[:, slot, :, :],
                                                      oprev[:, :, :])
                                nc.vector.copy_predicated(ostage[:, slot, :, :],
                                                          xb[u][:, 8:10, slot, :].bitcast(i32),
                                                          hstage[u][:, slot, :, :])
                    # flush staged h history + carries
                    for u in units:
                        rev = REV[u]
                        t0 = t0s[u]
                        if u != "2f":
                            nc.vector.tensor_copy(hs[u][:, ds(t0, STEPS_PER_BODY), :, :],
                                                  hstage[u][:, :, :, :])
                        last_slot = 0 if rev else STEPS_PER_BODY - 1
                        nc.vector.tensor_copy(hcar[u][:, :, :],
                                              hstage[u][:, last_slot, :, :])
                        if u == "2f":
                            nc.vector.tensor_copy(o2f_sb[:, ds(t0, STEPS_PER_BODY), :, :],
                                                  ostage[:, :, :, :])
                            nc.vector.tensor_copy(o2f_car[:, :, :],
                                                  ostage[:, STEPS_PER_BODY - 1, :, :])

        if stage in ("ph1", "xg2", "full"):
            for _rep in range(repeat):
                phase(("1f", "1b"), _rep)

        def l2_rhs(k, n):
            src = hs["1f"] if k < 2 else hs["1b"]
            return src[:, n * 64:(n + 1) * 64, k % 2, :]

        if stage in ("xg2", "full"):
            for _rep in range(repeat):
                xg_precompute("2f", l2_rhs, _rep)
                xg_precompute("2b", l2_rhs, _rep)

        if stage == "full":
            for _rep in range(repeat):
                phase(("2f", "2b"), _rep)

        # ---- classifier
        if stage != "full":
            with tc.tile_pool(name="dum", bufs=1) as dp:
                dmy = dp.tile([TAGS, 512], f32, tag="dmy")
                nc.vector.memset(dmy[:], 0.0)
                nc.sync.dma_start(logits_d[:, 0:512], dmy[:])
        else:
         with nc.named_scope("cls"), \
             tc.tile_pool(name="cl", bufs=3) as cp, \
             tc.tile_pool(name="cps", bufs=3, space="PSUM") as cps:
            for n in range(TB // 512):
                psm = cps.tile([TAGS, 512], f32, tag="ps")
                for k in range(4):
                    src = o2f_sb if k < 2 else hs["2b"]
                    nc.tensor.matmul(
                        out=psm[:],
                        lhsT=cls_sb[:, k * TAGS:(k + 1) * TAGS],
                        rhs=src[:, n * 64:(n + 1) * 64, k % 2, :],
                        start=(k == 0), stop=(k == 3))
                lg = cp.tile([TAGS, 512], f32, tag="lg")
                nc.vector.tensor_scalar_add(lg[:], psm[:], clsb_sb[:, :1])
                nc.sync.dma_start(logits_d[:, n * 512:(n + 1) * 512], lg[:])

    nc.compile()
    return nc


def _make_in_maps(inputs):
    words = np.asarray(inputs["words"]).astype(np.int32)
    lengths = np.asarray(inputs["lengths"]).astype(np.int32)
    emb = np.asarray(inputs["emb"], dtype=np.float32)
    mask = (lengths[:, None] > np.arange(T)[None, :]).astype(np.float32)
    wprep = {u: _prep_unit_weights(inputs[f"l{u}_Wih"], inputs[f"l{u}_Whh"],
                                   inputs[f"l{u}_bih"], inputs[f"l{u}_bhh"], MCNT[u])
             for u in UNITS}
    clsW = np.asarray(inputs["cls_W"], dtype=np.float32)
    CT = clsW.T
    clsx = np.concatenate([CT[k * 128:(k + 1) * 128, :] for k in range(4)],
                          axis=1).astype(ml_dtypes.bfloat16)
    clsb = np.asarray(inputs["cls_b"], dtype=np.float32).reshape(TAGS, 1)
    in_maps = []
    for c in range(NCORES):
        bsl = slice(c * Bc, (c + 1) * Bc)
        w_c = words[bsl]
        m_c = mask[bsl]
        words_tm = np.ascontiguousarray(w_c.T).reshape(TB, 1)
        aug = np.stack([(1.0 - m_c.T).reshape(TB), np.ones(TB, np.float32)]
                       ).astype(ml_dtypes.bfloat16)
        im = {"emb": emb, "words": words_tm, "aug": aug,
              "clsx": clsx, "clsb": clsb}
        for u in UNITS:
            wx, wa, wh = wprep[u]
            im[f"w{u}x"] = wx
            im[f"w{u}a"] = wa
            im[f"w{u}h"] = wh
        in_maps.append(im)
    return in_maps


def kernel(**inputs):
    if "nc" not in _CACHE:
        _CACHE["nc"] = _build_program()
    nc = _CACHE["nc"]
    in_maps = _make_in_maps(inputs)
    _CACHE["in_maps"] = in_maps
    res = run_bass_kernel_spmd(nc, in_maps, list(range(NCORES)))
    out = np.empty((B, T, TAGS), np.float32)
    for c in range(NCORES):
        lg = res.results[c]["logits"]          # [50, TB], col = t*Bc + b
        out[c * Bc:(c + 1) * Bc] = lg.reshape(TAGS, T, Bc).transpose(2, 1, 0)
    return out


def bench(inputs):
    """Run once with NTFF tracing; returns HW exec_time_ns (and stashes trace)."""
    kernel(**inputs)  # ensure program built/cached
    nc = _CACHE["nc"]
    in_maps = _CACHE["in_maps"]
    import tempfile
    tmpdir = tempfile.mkdtemp(prefix="bilstm_trace_")
    res = run_bass_kernel_spmd(nc, in_maps, list(range(NCORES)), trace=True,
                               tmpdir=tmpdir)
    _CACHE["trace_dir"] = tmpdir
    _CACHE["last_bench"] = res
    print("trace dir:", tmpdir)
    if res.per_core_scope_times:
        for scope, times in res.per_core_scope_times.items():
            print(f"scope {scope}: {times}")
    return res.exec_time_ns


if __name__ == "__main__":
    import reference
    inputs = {k: np.asarray(v) for k, v in reference.setup_inputs().items()}
    got = kernel(**inputs)
    print(got.shape, got.dtype)


# revision 13
# speedup vs baseline: 1.7548x; 1.0698x over previous
"""BiLSTM tagger on 8 TRN2 NeuronCores.

Strategy (hardcoded for B=64,T=512,V=30000,E=128,H=256,TAGS=50):
  - Data-parallel: batch sharded 8 ways (8 sequences/core); weights replicated.
  - Per core: embedding gather (indirect DMA) -> PE transpose -> x^T in SBUF;
    input projections xg = W_ih_aug @ [x; 1-m; 1] precomputed for all t as big
    matmuls into DRAM scratch; recurrences (l1 fwd+bwd step-interleaved, then
    l2 fwd+bwd) as dynamic Tile loops, 16 steps per iteration; classifier.
  - Masking: the (1-m) feature adds +/-60 to the f/i gate pre-activations at
    masked steps, freezing c exactly (sigmoid saturates to 1.0/0.0 in fp32).
    Backward-direction h is then exactly 0 at masked steps. Forward l2 output
    h is repaired with copy_predicated (hold previous output where mask=0).
    l1f's garbage masked outputs only feed masked l2 steps, where c is frozen.
  - Gate layout: gates on partitions, chunk order [i0 i1 f0 f1 g0 g1 o0 o1],
    batch on free dim. g rows pre-scaled by 2 so tanh(z)=2*sigmoid(2z)-1 lets
    one sigmoid cover i,f,g; per-step xg add done ON THE PE via an fp32
    identity matmul accumulating into PSUM (h-independent, issues early).
    PSUM split per step into main(i,f,g: 6 chunks) and o(2 chunks) banks so
    the main sigmoid doesn't wait for the o-gate matmuls.
"""
import sys

sys.path.insert(0, "/opt/trn_rl_repo")
import contextlib

import numpy as np
import ml_dtypes

import concourse.bass as bass
import concourse.bacc as bacc
import concourse.mybir as mybir
import concourse.tile as tile
from concourse.bass import ds
from concourse.bass_utils import run_bass_kernel_spmd
from concourse.masks import make_identity

B, T, V, E, H, TAGS = 64, 512, 30000, 128, 256, 50
NCORES = 8
Bc = B // NCORES          # 8 sequences per core
TB = T * Bc               # 4096 tokens per core
STEPS_PER_BODY = 16
NBODY = T // STEPS_PER_BODY  # 32

f32 = mybir.dt.float32
bf16 = mybir.dt.bfloat16
i32 = mybir.dt.int32

UNITS = ("1f", "1b", "2f", "2b")
KCNT = {"1f": 1, "1b": 1, "2f": 4, "2b": 4}       # 128-row K chunks of x features
MCNT = {"1f": 8, "1b": 8, "2f": 10, "2b": 8}      # 128-row output chunks
REV = {"1f": False, "1b": True, "2f": False, "2b": True}

_CACHE = {}


def _prep_unit_weights(Wih, Whh, bih, bhh, m_cnt):
    """Host-side weight marshalling for one LSTM direction.

    Row order already [i f g o]; g rows (512:768) scaled by 2 so that
    tanh(z) = 2*sigmoid(2z) - 1 can use the sigmoid table."""
    din = Wih.shape[1]
    Wp = np.asarray(Wih, np.float32).copy()          # [1024, din]
    Up = np.asarray(Whh, np.float32).copy()          # [1024, 256]
    bp = (np.asarray(bih) + np.asarray(bhh)).astype(np.float32)
    Wp[512:768] *= 2.0
    Up[512:768] *= 2.0
    bp = bp.copy()
    bp[512:768] *= 2.0
    M = m_cnt * 128
    k_cnt = din // 128
    # x-part lhsT: [din, M] -> k-chunk-major cols [128, k_cnt*M]
    WT = np.zeros((din, M), np.float32)
    WT[:, :1024] = Wp.T
    wx = np.concatenate([WT[k * 128:(k + 1) * 128, :] for k in range(k_cnt)],
                        axis=1).astype(ml_dtypes.bfloat16)  # [128, k_cnt*M]
    # aug lhsT rows: feature0 = (1-m), feature1 = 1
    wa = np.zeros((2, M), np.float32)
    wa[0, 0:256] = -60.0   # i rows: -60*(1-m)
    wa[0, 256:512] = 60.0  # f rows: +60*(1-m)
    wa[1, :1024] = bp
    if m_cnt == 10:        # l2f extra planes: m, m (for copy_predicated mask)
        wa[0, 1024:1280] = -1.0
        wa[1, 1024:1280] = 1.0
    wa = wa.astype(ml_dtypes.bfloat16)
    # Whh lhsT: [256, 1024] -> [128, 2*1024]
    UT = Up.T
    wh = np.concatenate([UT[0:128, :], UT[128:256, :]], axis=1).astype(ml_dtypes.bfloat16)
    return wx, wa, wh


def _build_program(stage="full", repeat=1):
    nc = bacc.Bacc("TRN2", target_bir_lowering=False, debug=False, num_devices=NCORES)
    emb_d = nc.dram_tensor("emb", [V, E], f32, kind="ExternalInput")
    words_d = nc.dram_tensor("words", [TB, 1], i32, kind="ExternalInput")
    aug_d = nc.dram_tensor("aug", [2, TB], bf16, kind="ExternalInput")
    wxd, wad, whd, xgd = {}, {}, {}, {}
    for u in UNITS:
        wxd[u] = nc.dram_tensor(f"w{u}x", [128, KCNT[u] * MCNT[u] * 128], bf16, kind="ExternalInput")
        wad[u] = nc.dram_tensor(f"w{u}a", [2, MCNT[u] * 128], bf16, kind="ExternalInput")
        whd[u] = nc.dram_tensor(f"w{u}h", [128, 2048], bf16, kind="ExternalInput")
        xgd[u] = nc.dram_tensor(f"xg{u}", [128, MCNT[u], T, Bc], f32)
    clsx_d = nc.dram_tensor("clsx", [128, 4 * TAGS], bf16, kind="ExternalInput")
    clsb_d = nc.dram_tensor("clsb", [TAGS, 1], f32, kind="ExternalInput")
    logits_d = nc.dram_tensor("logits", [TAGS, TB], f32, kind="ExternalOutput")

    ctx = contextlib.ExitStack()
    with tile.TileContext(nc) as tc, ctx:
        pp = ctx.enter_context(tc.tile_pool(name="persist", bufs=1))
        xT = pp.tile([128, TB], bf16, tag="xT")
        aug_sb = pp.tile([2, TB], bf16, tag="aug")
        ident = pp.tile([128, 128], f32, tag="ident")
        wx_sb = {u: pp.tile([128, KCNT[u] * MCNT[u] * 128], bf16, tag=f"wx{u}", name=f"wx{u}") for u in UNITS}
        wa_sb = {u: pp.tile([2, MCNT[u] * 128], bf16, tag=f"wa{u}", name=f"wa{u}") for u in UNITS}
        wh_sb = {u: pp.tile([128, 2048], bf16, tag=f"wh{u}", name=f"wh{u}") for u in UNITS}
        cls_sb = pp.tile([128, 4 * TAGS], bf16, tag="clsx")
        clsb_sb = pp.tile([TAGS, 1], f32, tag="clsb")
        hs = {u: pp.tile([128, T, 2, Bc], bf16, tag=f"hs{u}", name=f"hs{u}")
              for u in ("1f", "1b", "2b")}
        o2f_sb = pp.tile([128, T, 2, Bc], bf16, tag="o2f")
        hcar = {u: pp.tile([128, 2, Bc], bf16, tag=f"hc{u}", name=f"hc{u}") for u in UNITS}
        ccar = {u: pp.tile([128, 2, Bc], f32, tag=f"cc{u}", name=f"cc{u}") for u in UNITS}
        o2f_run = pp.tile([128, 2, Bc], bf16, tag="o2fr")

        # ---- load weights / constants
        make_identity(nc, ident[:])
        for u in UNITS:
            nc.sync.dma_start(wx_sb[u][:], wxd[u][:])
            nc.sync.dma_start(wa_sb[u][:], wad[u][:])
            nc.sync.dma_start(wh_sb[u][:], whd[u][:])
        nc.sync.dma_start(cls_sb[:], clsx_d[:])
        nc.sync.dma_start(clsb_sb[:], clsb_d[:])
        nc.sync.dma_start(aug_sb[:], aug_d[:])
        for u in UNITS:
            nc.vector.memset(hcar[u][:, :, :], 0.0)
            nc.vector.memset(ccar[u][:, :, :], 0.0)
        nc.vector.memset(o2f_run[:, :, :], 0.0)

        # ---- embedding gather + transpose into xT
        for _rep in range(repeat):
         with nc.named_scope("gather"), \
             tc.tile_pool(name=f"gat{_rep}", bufs=3) as gp, \
             tc.tile_pool(name=f"gps{_rep}", bufs=3, space="PSUM") as gps:
            for n in range(TB // 128):
                idx = gp.tile([128, 1], i32, tag="idx")
                nc.sync.dma_start(idx[:], words_d[n * 128:(n + 1) * 128, :])
                xt = gp.tile([128, 128], f32, tag="xt")
                nc.gpsimd.indirect_dma_start(
                    out=xt[:], out_offset=None, in_=emb_d[:, :],
                    in_offset=bass.IndirectOffsetOnAxis(ap=idx[:, :1], axis=0))
                pst = gps.tile([128, 128], f32, tag="pst")
                nc.tensor.transpose(out=pst[:], in_=xt[:], identity=ident[:])
                nc.vector.tensor_copy(xT[:, n * 128:(n + 1) * 128], pst[:])

        # ---- xg precompute into DRAM, layout [128, m, T, Bc]
        def xg_precompute(u, rhs_of_k, _rep=0):
            m_cnt, k_cnt = MCNT[u], KCNT[u]
            with nc.named_scope(f"xg{u}"), \
                 tc.tile_pool(name=f"xp{u}{_rep}", bufs=4, space="PSUM") as xps, \
                 tc.tile_pool(name=f"xs{u}{_rep}", bufs=4) as xsb:
                for n in range(TB // 512):
                    nsl = slice(n * 512, (n + 1) * 512)
                    for m in range(m_cnt):
                        psm = xps.tile([128, 512], f32, tag="ps")
                        first = True
                        if m < 8:  # gate chunks get the x contribution
                            for k in range(k_cnt):
                                nc.tensor.matmul(
                                    out=psm[:],
                                    lhsT=wx_sb[u][:, (k * m_cnt + m) * 128:(k * m_cnt + m + 1) * 128],
                                    rhs=rhs_of_k(k, n),
                                    start=first, stop=False)
                                first = False
                        nc.tensor.matmul(
                            out=psm[:],
                            lhsT=wa_sb[u][:, m * 128:(m + 1) * 128],
                            rhs=aug_sb[:, nsl],
                            start=first, stop=True)
                        stg = xsb.tile([128, 512], f32, tag="stg")
                        if (n + m) % 2 == 0:
                            nc.vector.tensor_copy(stg[:], psm[:])
                        else:
                            nc.scalar.activation(stg[:], psm[:],
                                                 mybir.ActivationFunctionType.Copy)
                        nc.sync.dma_start(
                            xgd[u][:, m, n * 64:(n + 1) * 64, :],
                            stg[:, :].rearrange("p (t b) -> p t b", b=Bc))

        def l1_rhs(k, n):
            return xT[:, n * 512:(n + 1) * 512]

        if stage != "gather":
            for _rep in range(repeat):
                xg_precompute("1f", l1_rhs, _rep)
                xg_precompute("1b", l1_rhs, _rep)

        # ---- recurrence phase: the two direction units step-interleaved
        def phase(units, _rep=0):
            with nc.named_scope(f"ph{units[0]}"), \
                 tc.tile_pool(name=f"rc{units[0]}{_rep}", bufs=2) as rp, \
                 tc.tile_pool(name=f"rps{units[0]}{_rep}", bufs=3, space="PSUM") as rps, \
                 tc.tile_pool(name=f"rtmp{units[0]}{_rep}", bufs=3) as tp:
                with tc.For_i(0, NBODY, staggered_reset=True,
                              hint_engines=(mybir.EngineType.PE,
                                            mybir.EngineType.Activation,
                                            mybir.EngineType.DVE)) as i:
                    xb, hstage, t0s = {}, {}, {}
                    ostage = None
                    for u in units:
                        m_cnt = MCNT[u]
                        if REV[u]:
                            t0 = i * (-STEPS_PER_BODY) + (T - STEPS_PER_BODY)
                        else:
                            t0 = i * STEPS_PER_BODY
                        t0s[u] = t0
                        xb[u] = rp.tile([128, m_cnt, STEPS_PER_BODY, Bc], f32,
                                        tag=f"xb{u}", name=f"xb{u}")
                        nc.sync.dma_start(xb[u][:, :, :, :],
                                          xgd[u][:, :, ds(t0, STEPS_PER_BODY), :])
                        hstage[u] = rp.tile([128, STEPS_PER_BODY, 2, Bc], bf16,
                                            tag=f"hst{u}", name=f"hst{u}")
                        if u == "2f":
                            ostage = rp.tile([128, STEPS_PER_BODY, 2, Bc], bf16,
                                             tag="ost")
                    for us in range(STEPS_PER_BODY):
                        for u in units:
                            rev = REV[u]
                            slot = (STEPS_PER_BODY - 1 - us) if rev else us
                            if us == 0:
                                hprev = hcar[u]
                            else:
                                pslot = slot + 1 if rev else slot - 1
                                hprev = hstage[u][:, pslot, :, :]
                            psm = rps.tile([128, 8, Bc], f32, tag=f"g{u}")
                            # xg add on PE (h-independent, issues early)
                            nc.tensor.matmul(out=psm[:, :, :], lhsT=ident[:],
                                             rhs=xb[u][:, 0:8, slot, :],
                                             start=True, stop=False)
                            for m in range(8):
                                for k in range(2):
                                    nc.tensor.matmul(
                                        out=psm[:, m, :],
                                        lhsT=wh_sb[u][:, (k * 8 + m) * 128:(k * 8 + m + 1) * 128],
                                        rhs=hprev[:, k, :],
                                        start=False, stop=(m == 7 and k == 1))
                            sg = tp.tile([128, 8, Bc], f32, tag=f"sg{u}")
                            nc.scalar.activation(sg[:, :, :], psm[:, :, :],
                                                 mybir.ActivationFunctionType.Sigmoid)
                            csf = tp.tile([128, 2, Bc], f32, tag=f"csf{u}")
                            nc.vector.tensor_tensor(out=csf[:, :, :], in0=sg[:, 2:4, :],
                                                    in1=ccar[u][:, :, :], op=mybir.AluOpType.mult)
                            gt = tp.tile([128, 2, Bc], f32, tag=f"gt{u}")
                            nc.vector.tensor_scalar(out=gt[:, :, :], in0=sg[:, 4:6, :],
                                                    scalar1=2.0, scalar2=-1.0,
                                                    op0=mybir.AluOpType.mult,
                                                    op1=mybir.AluOpType.add)
                            t1 = tp.tile([128, 2, Bc], f32, tag=f"t1{u}")
                            nc.vector.tensor_tensor(out=t1[:, :, :], in0=sg[:, 0:2, :],
                                                    in1=gt[:, :, :], op=mybir.AluOpType.mult)
                            nc.vector.tensor_tensor(out=ccar[u][:, :, :], in0=csf[:, :, :],
                                                    in1=t1[:, :, :], op=mybir.AluOpType.add)
                            tc2 = tp.tile([128, 2, Bc], f32, tag=f"tc2{u}")
                            nc.scalar.activation(tc2[:, :, :], ccar[u][:, :, :],
                                                 mybir.ActivationFunctionType.Tanh)
                            nc.vector.tensor_tensor(out=hstage[u][:, slot, :, :],
                                                    in0=sg[:, 6:8, :],
                                                    in1=tc2[:, :, :], op=mybir.AluOpType.mult)
                            if u == "2f":
                                # running masked output: keep prev where mask=0
                                nc.vector.copy_predicated(o2f_run[:, :, :],
                                                          xb[u][:, 8:10, slot, :].bitcast(i32),
                                                          hstage[u][:, slot, :, :])
                                nc.gpsimd.tensor_copy(ostage[:, slot, :, :],
                                                      o2f_run[:, :, :])
                    # flush staged h history + carries (on idle GpSimd)
                    for u in units:
                        rev = REV[u]
                        t0 = t0s[u]
                        if u != "2f":
                            nc.gpsimd.tensor_copy(hs[u][:, ds(t0, STEPS_PER_BODY), :, :],
                                                  hstage[u][:, :, :, :])
                        last_slot = 0 if rev else STEPS_PER_BODY - 1
                        nc.gpsimd.tensor_copy(hcar[u][:, :, :],
                                              hstage[u][:, last_slot, :, :])
                        if u == "2f":
                            nc.gpsimd.tensor_copy(o2f_sb[:, ds(t0, STEPS_PER_BODY), :, :],
                                                  ostage[:, :, :, :])

        if stage in ("ph1", "xg2", "full"):
            for _rep in range(repeat):
                phase(("1f", "1b"), _rep)

        def l2_rhs(k, n):
            src = hs["1f"] if k < 2 else hs["1b"]
            return src[:, n * 64:(n + 1) * 64, k % 2, :]

        if stage in ("xg2", "full"):
            for _rep in range(repeat):
                xg_precompute("2f", l2_rhs, _rep)
                xg_precompute("2b", l2_rhs, _rep)

        if stage == "full":
            for _rep in range(repeat):
                phase(("2f", "2b"), _rep)

        # ---- classifier
        if stage != "full":
            with tc.tile_pool(name="dum", bufs=1) as dp:
                dmy = dp.tile([TAGS, 512], f32, tag="dmy")
                nc.vector.memset(dmy[:], 0.0)
                nc.sync.dma_start(logits_d[:, 0:512], dmy[:])
        else:
         with nc.named_scope("cls"), \
             tc.tile_pool(name="cl", bufs=3) as cp, \
             tc.tile_pool(name="cps", bufs=3, space="PSUM") as cps:
            for n in range(TB // 512):
                psm = cps.tile([TAGS, 512], f32, tag="ps")
                for k in range(4):
                    src = o2f_sb if k < 2 else hs["2b"]
                    nc.tensor.matmul(
                        out=psm[:],
                        lhsT=cls_sb[:, k * TAGS:(k + 1) * TAGS],
                        rhs=src[:, n * 64:(n + 1) * 64, k % 2, :],
                        start=(k == 0), stop=(k == 3))
                lg = cp.tile([TAGS, 512], f32, tag="lg")
                nc.vector.tensor_scalar_add(lg[:], psm[:], clsb_sb[:, :1])
                nc.sync.dma_start(logits_d[:, n * 512:(n + 1) * 512], lg[:])

    nc.compile()
    return nc


def _make_in_maps(inputs):
    words = np.asarray(inputs["words"]).astype(np.int32)
    lengths = np.asarray(inputs["lengths"]).astype(np.int32)
    emb = np.asarray(inputs["emb"], dtype=np.float32)
    mask = (lengths[:, None] > np.arange(T)[None, :]).astype(np.float32)
    wprep = {u: _prep_unit_weights(inputs[f"l{u}_Wih"], inputs[f"l{u}_Whh"],
                                   inputs[f"l{u}_bih"], inputs[f"l{u}_bhh"], MCNT[u])
             for u in UNITS}
    clsW = np.asarray(inputs["cls_W"], dtype=np.float32)
    CT = clsW.T
    clsx = np.concatenate([CT[k * 128:(k + 1) * 128, :] for k in range(4)],
                          axis=1).astype(ml_dtypes.bfloat16)
    clsb = np.asarray(inputs["cls_b"], dtype=np.float32).reshape(TAGS, 1)
    in_maps = []
    for c in range(NCORES):
        bsl = slice(c * Bc, (c + 1) * Bc)
        w_c = words[bsl]
        m_c = mask[bsl]
        words_tm = np.ascontiguousarray(w_c.T).reshape(TB, 1)
        aug = np.stack([(1.0 - m_c.T).reshape(TB), np.ones(TB, np.float32)]
                       ).astype(ml_dtypes.bfloat16)
        im = {"emb": emb, "words": words_tm, "aug": aug,
              "clsx": clsx, "clsb": clsb}
        for u in UNITS:
            wx, wa, wh = wprep[u]
            im[f"w{u}x"] = wx
            im[f"w{u}a"] = wa
            im[f"w{u}h"] = wh
        in_maps.append(im)
    return in_maps


def kernel(**inputs):
    if "nc" not in _CACHE:
        _CACHE["nc"] = _build_program()
    nc = _CACHE["nc"]
    in_maps = _make_in_maps(inputs)
    _CACHE["in_maps"] = in_maps
    res = run_bass_kernel_spmd(nc, in_maps, list(range(NCORES)))
    out = np.empty((B, T, TAGS), np.float32)
    for c in range(NCORES):
        lg = res.results[c]["logits"]          # [50, TB], col = t*Bc + b
        out[c * Bc:(c + 1) * Bc] = lg.reshape(TAGS, T, Bc).transpose(2, 1, 0)
    return out


def bench(inputs):
    """Run once with NTFF tracing; returns HW exec_time_ns (and stashes trace)."""
    kernel(**inputs)  # ensure program built/cached
    nc = _CACHE["nc"]
    in_maps = _CACHE["in_maps"]
    import tempfile
    tmpdir = tempfile.mkdtemp(prefix="bilstm_trace_")
    res = run_bass_kernel_spmd(nc, in_maps, list(range(NCORES)), trace=True,
                               tmpdir=tmpdir)
    _CACHE["trace_dir"] = tmpdir
    _CACHE["last_bench"] = res
    print("trace dir:", tmpdir)
    if res.per_core_scope_times:
        for scope, times in res.per_core_scope_times.items():
            print(f"scope {scope}: {times}")
    return res.exec_time_ns


if __name__ == "__main__":
    import reference
    inputs = {k: np.asarray(v) for k, v in reference.setup_inputs().items()}
    got = kernel(**inputs)
    print(got.shape, got.dtype)


# revision 17
# speedup vs baseline: 1.7686x; 1.0079x over previous
"""BiLSTM tagger on 8 TRN2 NeuronCores.

Strategy (hardcoded for B=64,T=512,V=30000,E=128,H=256,TAGS=50):
  - Data-parallel: batch sharded 8 ways (8 sequences/core); weights replicated.
  - Per core: embedding gather (indirect DMA) -> PE transpose -> x^T in SBUF;
    input projections xg = W_ih_aug @ [x; 1-m; 1] precomputed for all t as big
    matmuls into DRAM scratch; recurrences (l1 fwd+bwd step-interleaved, then
    l2 fwd+bwd) as dynamic Tile loops, 16 steps per iteration; classifier.
  - Masking: the (1-m) feature adds +/-60 to the f/i gate pre-activations at
    masked steps, freezing c exactly (sigmoid saturates to 1.0/0.0 in fp32).
    Backward-direction h is then exactly 0 at masked steps. Forward l2 output
    h is repaired with copy_predicated (hold previous output where mask=0).
    l1f's garbage masked outputs only feed masked l2 steps, where c is frozen.
  - Gate layout: gates on partitions, chunk order [i0 i1 f0 f1 g0 g1 o0 o1],
    batch on free dim. g rows pre-scaled by 2 so tanh(z)=2*sigmoid(2z)-1 lets
    one sigmoid cover i,f,g; per-step xg add done ON THE PE via an fp32
    identity matmul accumulating into PSUM (h-independent, issues early).
    PSUM split per step into main(i,f,g: 6 chunks) and o(2 chunks) banks so
    the main sigmoid doesn't wait for the o-gate matmuls.
"""
import sys

sys.path.insert(0, "/opt/trn_rl_repo")
import contextlib

import numpy as np
import ml_dtypes

import concourse.bass as bass
import concourse.bacc as bacc
import concourse.mybir as mybir
import concourse.tile as tile
from concourse.bass import ds
from concourse.bass_utils import run_bass_kernel_spmd
from concourse.masks import make_identity

B, T, V, E, H, TAGS = 64, 512, 30000, 128, 256, 50
NCORES = 8
Bc = B // NCORES          # 8 sequences per core
TB = T * Bc               # 4096 tokens per core
STEPS_PER_BODY = 32
NBODY = T // STEPS_PER_BODY  # 16

f32 = mybir.dt.float32
bf16 = mybir.dt.bfloat16
i32 = mybir.dt.int32

UNITS = ("1f", "1b", "2f", "2b")
KCNT = {"1f": 1, "1b": 1, "2f": 4, "2b": 4}       # 128-row K chunks of x features
MCNT = {"1f": 8, "1b": 8, "2f": 10, "2b": 8}      # 128-row output chunks
REV = {"1f": False, "1b": True, "2f": False, "2b": True}

_CACHE = {}


PERM = np.concatenate([np.arange(0, 512), np.arange(768, 1024),
                       np.arange(512, 768)])  # chunk order [i0 i1 f0 f1 o0 o1 g0 g1]


def _prep_unit_weights(Wih, Whh, bih, bhh, m_cnt):
    """Host-side weight marshalling for one LSTM direction.

    Torch row order [i f g o] permuted to chunk order [i i f f o o g g] so
    sigmoid covers contiguous chunks 0:6 and tanh chunks 6:8."""
    din = Wih.shape[1]
    Wp = np.asarray(Wih, np.float32)[PERM]           # [1024, din]
    Up = np.asarray(Whh, np.float32)[PERM]           # [1024, 256]
    bp = (np.asarray(bih) + np.asarray(bhh)).astype(np.float32)[PERM]
    M = m_cnt * 128
    k_cnt = din // 128
    # x-part lhsT: [din, M] -> k-chunk-major cols [128, k_cnt*M]
    WT = np.zeros((din, M), np.float32)
    WT[:, :1024] = Wp.T
    wx = np.concatenate([WT[k * 128:(k + 1) * 128, :] for k in range(k_cnt)],
                        axis=1).astype(ml_dtypes.bfloat16)  # [128, k_cnt*M]
    # aug lhsT rows: feature0 = (1-m), feature1 = 1
    wa = np.zeros((2, M), np.float32)
    wa[0, 0:256] = -60.0   # i rows: -60*(1-m)
    wa[0, 256:512] = 60.0  # f rows: +60*(1-m)
    wa[1, :1024] = bp
    if m_cnt == 10:        # l2f extra planes: m, m (for copy_predicated mask)
        wa[0, 1024:1280] = -1.0
        wa[1, 1024:1280] = 1.0
    wa = wa.astype(ml_dtypes.bfloat16)
    # Whh lhsT: [256, 1024] -> [128, 2*1024]
    UT = Up.T
    wh = np.concatenate([UT[0:128, :], UT[128:256, :]], axis=1).astype(ml_dtypes.bfloat16)
    return wx, wa, wh


def _build_program(stage="full", repeat=1):
    nc = bacc.Bacc("TRN2", target_bir_lowering=False, debug=False, num_devices=NCORES)
    emb_d = nc.dram_tensor("emb", [V, E], f32, kind="ExternalInput")
    words_d = nc.dram_tensor("words", [TB, 1], i32, kind="ExternalInput")
    aug_d = nc.dram_tensor("aug", [2, TB], bf16, kind="ExternalInput")
    wxd, wad, whd, xgd = {}, {}, {}, {}
    for u in UNITS:
        wxd[u] = nc.dram_tensor(f"w{u}x", [128, KCNT[u] * MCNT[u] * 128], bf16, kind="ExternalInput")
        wad[u] = nc.dram_tensor(f"w{u}a", [2, MCNT[u] * 128], bf16, kind="ExternalInput")
        whd[u] = nc.dram_tensor(f"w{u}h", [128, 2048], bf16, kind="ExternalInput")
        xgd[u] = nc.dram_tensor(f"xg{u}", [128, MCNT[u], T, Bc], f32)
    clsx_d = nc.dram_tensor("clsx", [128, 4 * TAGS], bf16, kind="ExternalInput")
    clsb_d = nc.dram_tensor("clsb", [TAGS, 1], f32, kind="ExternalInput")
    logits_d = nc.dram_tensor("logits", [TAGS, TB], f32, kind="ExternalOutput")

    ctx = contextlib.ExitStack()
    with tile.TileContext(nc) as tc, ctx:
        pp = ctx.enter_context(tc.tile_pool(name="persist", bufs=1))
        xT = pp.tile([128, TB], bf16, tag="xT")
        aug_sb = pp.tile([2, TB], bf16, tag="aug")
        ident = pp.tile([128, 128], f32, tag="ident")
        wx_sb = {u: pp.tile([128, KCNT[u] * MCNT[u] * 128], bf16, tag=f"wx{u}", name=f"wx{u}") for u in UNITS}
        wa_sb = {u: pp.tile([2, MCNT[u] * 128], bf16, tag=f"wa{u}", name=f"wa{u}") for u in UNITS}
        wh_sb = {u: pp.tile([128, 2048], bf16, tag=f"wh{u}", name=f"wh{u}") for u in UNITS}
        cls_sb = pp.tile([128, 4 * TAGS], bf16, tag="clsx")
        clsb_sb = pp.tile([TAGS, 1], f32, tag="clsb")
        hs = {u: pp.tile([128, T, 2, Bc], bf16, tag=f"hs{u}", name=f"hs{u}")
              for u in ("1f", "1b", "2b")}
        o2f_sb = pp.tile([128, T, 2, Bc], bf16, tag="o2f")
        hcar = {u: pp.tile([128, 2, Bc], bf16, tag=f"hc{u}", name=f"hc{u}") for u in UNITS}
        ccar = {u: pp.tile([128, 2, Bc], f32, tag=f"cc{u}", name=f"cc{u}") for u in UNITS}
        o2f_run = pp.tile([128, 2, Bc], bf16, tag="o2fr")

        # ---- load weights / constants
        make_identity(nc, ident[:])
        for u in UNITS:
            nc.sync.dma_start(wx_sb[u][:], wxd[u][:])
            nc.sync.dma_start(wa_sb[u][:], wad[u][:])
            nc.sync.dma_start(wh_sb[u][:], whd[u][:])
        nc.sync.dma_start(cls_sb[:], clsx_d[:])
        nc.sync.dma_start(clsb_sb[:], clsb_d[:])
        nc.sync.dma_start(aug_sb[:], aug_d[:])
        for u in UNITS:
            nc.vector.memset(hcar[u][:, :, :], 0.0)
            nc.vector.memset(ccar[u][:, :, :], 0.0)
        nc.vector.memset(o2f_run[:, :, :], 0.0)

        # ---- embedding gather + transpose into xT
        for _rep in range(repeat):
         with nc.named_scope("gather"), \
             tc.tile_pool(name=f"gat{_rep}", bufs=3) as gp, \
             tc.tile_pool(name=f"gps{_rep}", bufs=3, space="PSUM") as gps:
            for n in range(TB // 128):
                idx = gp.tile([128, 1], i32, tag="idx")
                nc.sync.dma_start(idx[:], words_d[n * 128:(n + 1) * 128, :])
                xt = gp.tile([128, 128], f32, tag="xt")
                nc.gpsimd.indirect_dma_start(
                    out=xt[:], out_offset=None, in_=emb_d[:, :],
                    in_offset=bass.IndirectOffsetOnAxis(ap=idx[:, :1], axis=0))
                pst = gps.tile([128, 128], f32, tag="pst")
                nc.tensor.transpose(out=pst[:], in_=xt[:], identity=ident[:])
                nc.vector.tensor_copy(xT[:, n * 128:(n + 1) * 128], pst[:])

        # ---- xg precompute into DRAM, layout [128, m, T, Bc]
        def xg_precompute(u, rhs_of_k, _rep=0):
            m_cnt, k_cnt = MCNT[u], KCNT[u]
            with nc.named_scope(f"xg{u}"), \
                 tc.tile_pool(name=f"xp{u}{_rep}", bufs=4, space="PSUM") as xps, \
                 tc.tile_pool(name=f"xs{u}{_rep}", bufs=4) as xsb:
                for n in range(TB // 512):
                    nsl = slice(n * 512, (n + 1) * 512)
                    for m in range(m_cnt):
                        psm = xps.tile([128, 512], f32, tag="ps")
                        first = True
                        if m < 8:  # gate chunks get the x contribution
                            for k in range(k_cnt):
                                nc.tensor.matmul(
                                    out=psm[:],
                                    lhsT=wx_sb[u][:, (k * m_cnt + m) * 128:(k * m_cnt + m + 1) * 128],
                                    rhs=rhs_of_k(k, n),
                                    start=first, stop=False)
                                first = False
                        nc.tensor.matmul(
                            out=psm[:],
                            lhsT=wa_sb[u][:, m * 128:(m + 1) * 128],
                            rhs=aug_sb[:, nsl],
                            start=first, stop=True)
                        stg = xsb.tile([128, 512], f32, tag="stg")
                        if (n + m) % 2 == 0:
                            nc.vector.tensor_copy(stg[:], psm[:])
                        else:
                            nc.scalar.activation(stg[:], psm[:],
                                                 mybir.ActivationFunctionType.Copy)
                        nc.sync.dma_start(
                            xgd[u][:, m, n * 64:(n + 1) * 64, :],
                            stg[:, :].rearrange("p (t b) -> p t b", b=Bc))

        def l1_rhs(k, n):
            return xT[:, n * 512:(n + 1) * 512]

        if stage != "gather":
            for _rep in range(repeat):
                xg_precompute("1f", l1_rhs, _rep)
                xg_precompute("1b", l1_rhs, _rep)

        # ---- recurrence phase: the two direction units step-interleaved
        def phase(units, _rep=0):
            with nc.named_scope(f"ph{units[0]}"), \
                 tc.tile_pool(name=f"rc{units[0]}{_rep}", bufs=2) as rp, \
                 tc.tile_pool(name=f"rps{units[0]}{_rep}", bufs=2, space="PSUM") as rps, \
                 tc.tile_pool(name=f"rtmp{units[0]}{_rep}", bufs=3) as tp:
                with tc.For_i(0, NBODY, staggered_reset=True,
                              hint_engines=(mybir.EngineType.PE,
                                            mybir.EngineType.Activation,
                                            mybir.EngineType.DVE)) as i:
                    xb, hstage, t0s = {}, {}, {}
                    ostage = None
                    for u in units:
                        m_cnt = MCNT[u]
                        if REV[u]:
                            t0 = i * (-STEPS_PER_BODY) + (T - STEPS_PER_BODY)
                        else:
                            t0 = i * STEPS_PER_BODY
                        t0s[u] = t0
                        xb[u] = rp.tile([128, m_cnt, STEPS_PER_BODY, Bc], f32,
                                        tag=f"xb{u}", name=f"xb{u}")
                        nc.sync.dma_start(xb[u][:, :, :, :],
                                          xgd[u][:, :, ds(t0, STEPS_PER_BODY), :])
                        hstage[u] = rp.tile([128, STEPS_PER_BODY, 2, Bc], bf16,
                                            tag=f"hst{u}", name=f"hst{u}")
                        if u == "2f":
                            ostage = rp.tile([128, STEPS_PER_BODY, 2, Bc], bf16,
                                             tag="ost")
                    for us in range(STEPS_PER_BODY):
                        for u in units:
                            rev = REV[u]
                            slot = (STEPS_PER_BODY - 1 - us) if rev else us
                            if us == 0:
                                hprev = hcar[u]
                            else:
                                pslot = slot + 1 if rev else slot - 1
                                hprev = hstage[u][:, pslot, :, :]
                            # separate PSUM banks: sigmoid chunks [i,f,o] and
                            # tanh chunks [g] so sigmoid needn't wait on g MMs
                            psm = rps.tile([128, 6, Bc], f32, tag=f"g{u}")
                            psg = rps.tile([128, 2, Bc], f32, tag=f"gg{u}")
                            # xg add on PE (h-independent, issues early)
                            nc.tensor.matmul(out=psm[:, :, :], lhsT=ident[:],
                                             rhs=xb[u][:, 0:6, slot, :],
                                             start=True, stop=False)
                            nc.tensor.matmul(out=psg[:, :, :], lhsT=ident[:],
                                             rhs=xb[u][:, 6:8, slot, :],
                                             start=True, stop=False)
                            for m in range(6):
                                for k in range(2):
                                    nc.tensor.matmul(
                                        out=psm[:, m, :],
                                        lhsT=wh_sb[u][:, (k * 8 + m) * 128:(k * 8 + m + 1) * 128],
                                        rhs=hprev[:, k, :],
                                        start=False, stop=(m == 5 and k == 1))
                            for m in range(6, 8):
                                for k in range(2):
                                    nc.tensor.matmul(
                                        out=psg[:, m - 6, :],
                                        lhsT=wh_sb[u][:, (k * 8 + m) * 128:(k * 8 + m + 1) * 128],
                                        rhs=hprev[:, k, :],
                                        start=False, stop=(m == 7 and k == 1))
                            sg = tp.tile([128, 6, Bc], f32, tag=f"sg{u}")
                            nc.scalar.activation(sg[:, :, :], psm[:, :, :],
                                                 mybir.ActivationFunctionType.Sigmoid)
                            tg = tp.tile([128, 2, Bc], f32, tag=f"tg{u}")
                            nc.scalar.activation(tg[:, :, :], psg[:, :, :],
                                                 mybir.ActivationFunctionType.Tanh)
                            csf = tp.tile([128, 2, Bc], f32, tag=f"csf{u}")
                            nc.gpsimd.tensor_tensor(out=csf[:, :, :], in0=sg[:, 2:4, :],
                                                    in1=ccar[u][:, :, :], op=mybir.AluOpType.mult)
                            t1 = tp.tile([128, 2, Bc], f32, tag=f"t1{u}")
                            nc.vector.tensor_tensor(out=t1[:, :, :], in0=sg[:, 0:2, :],
                                                    in1=tg[:, :, :], op=mybir.AluOpType.mult)
                            nc.vector.tensor_tensor(out=ccar[u][:, :, :], in0=csf[:, :, :],
                                                    in1=t1[:, :, :], op=mybir.AluOpType.add)
                            tc2 = tp.tile([128, 2, Bc], f32, tag=f"tc2{u}")
                            nc.scalar.activation(tc2[:, :, :], ccar[u][:, :, :],
                                                 mybir.ActivationFunctionType.Tanh)
                            nc.vector.tensor_tensor(out=hstage[u][:, slot, :, :],
                                                    in0=sg[:, 4:6, :],
                                                    in1=tc2[:, :, :], op=mybir.AluOpType.mult)
                            if u == "2f":
                                # running masked output: keep prev where mask=0
                                nc.vector.copy_predicated(o2f_run[:, :, :],
                                                          xb[u][:, 8:10, slot, :].bitcast(i32),
                                                          hstage[u][:, slot, :, :])
                                nc.gpsimd.tensor_copy(ostage[:, slot, :, :],
                                                      o2f_run[:, :, :])
                    # flush staged h history + carries (on idle GpSimd)
                    for u in units:
                        rev = REV[u]
                        t0 = t0s[u]
                        if u != "2f":
                            nc.gpsimd.tensor_copy(hs[u][:, ds(t0, STEPS_PER_BODY), :, :],
                                                  hstage[u][:, :, :, :])
                        last_slot = 0 if rev else STEPS_PER_BODY - 1
                        nc.gpsimd.tensor_copy(hcar[u][:, :, :],
                                              hstage[u][:, last_slot, :, :])
                        if u == "2f":
                            nc.gpsimd.tensor_copy(o2f_sb[:, ds(t0, STEPS_PER_BODY), :, :],
                                                  ostage[:, :, :, :])

        if stage in ("ph1", "xg2", "full"):
            for _rep in range(repeat):
                phase(("1f", "1b"), _rep)

        def l2_rhs(k, n):
            src = hs["1f"] if k < 2 else hs["1b"]
            return src[:, n * 64:(n + 1) * 64, k % 2, :]

        if stage in ("xg2", "full"):
            for _rep in range(repeat):
                xg_precompute("2f", l2_rhs, _rep)
                xg_precompute("2b", l2_rhs, _rep)

        if stage == "full":
            for _rep in range(repeat):
                phase(("2f", "2b"), _rep)

        # ---- classifier
        if stage != "full":
            with tc.tile_pool(name="dum", bufs=1) as dp:
                dmy = dp.tile([TAGS, 512], f32, tag="dmy")
                nc.vector.memset(dmy[:], 0.0)
                nc.sync.dma_start(logits_d[:, 0:512], dmy[:])
        else:
         with nc.named_scope("cls"), \
             tc.tile_pool(name="cl", bufs=3) as cp, \
             tc.tile_pool(name="cps", bufs=3, space="PSUM") as cps:
            for n in range(TB // 512):
                psm = cps.tile([TAGS, 512], f32, tag="ps")
                for k in range(4):
                    src = o2f_sb if k < 2 else hs["2b"]
                    nc.tensor.matmul(
                        out=psm[:],
                        lhsT=cls_sb[:, k * TAGS:(k + 1) * TAGS],
                        rhs=src[:, n * 64:(n + 1) * 64, k % 2, :],
                        start=(k == 0), stop=(k == 3))
                lg = cp.tile([TAGS, 512], f32, tag="lg")
                nc.vector.tensor_scalar_add(lg[:], psm[:], clsb_sb[:, :1])
                nc.sync.dma_start(logits_d[:, n * 512:(n + 1) * 512], lg[:])

    nc.compile()
    return nc


def _make_in_maps(inputs):
    words = np.asarray(inputs["words"]).astype(np.int32)
    lengths = np.asarray(inputs["lengths"]).astype(np.int32)
    emb = np.asarray(inputs["emb"], dtype=np.float32)
    mask = (lengths[:, None] > np.arange(T)[None, :]).astype(np.float32)
    wprep = {u: _prep_unit_weights(inputs[f"l{u}_Wih"], inputs[f"l{u}_Whh"],
                                   inputs[f"l{u}_bih"], inputs[f"l{u}_bhh"], MCNT[u])
             for u in UNITS}
    clsW = np.asarray(inputs["cls_W"], dtype=np.float32)
    CT = clsW.T
    clsx = np.concatenate([CT[k * 128:(k + 1) * 128, :] for k in range(4)],
                          axis=1).astype(ml_dtypes.bfloat16)
    clsb = np.asarray(inputs["cls_b"], dtype=np.float32).reshape(TAGS, 1)
    in_maps = []
    for c in range(NCORES):
        bsl = slice(c * Bc, (c + 1) * Bc)
        w_c = words[bsl]
        m_c = mask[bsl]
        words_tm = np.ascontiguousarray(w_c.T).reshape(TB, 1)
        aug = np.stack([(1.0 - m_c.T).reshape(TB), np.ones(TB, np.float32)]
                       ).astype(ml_dtypes.bfloat16)
        im = {"emb": emb, "words": words_tm, "aug": aug,
              "clsx": clsx, "clsb": clsb}
        for u in UNITS:
            wx, wa, wh = wprep[u]
            im[f"w{u}x"] = wx
            im[f"w{u}a"] = wa
            im[f"w{u}h"] = wh
        in_maps.append(im)
    return in_maps


def kernel(**inputs):
    if "nc" not in _CACHE:
        _CACHE["nc"] = _build_program()
    nc = _CACHE["nc"]
    in_maps = _make_in_maps(inputs)
    _CACHE["in_maps"] = in_maps
    res = run_bass_kernel_spmd(nc, in_maps, list(range(NCORES)))
    out = np.empty((B, T, TAGS), np.float32)
    for c in range(NCORES):
        lg = res.results[c]["logits"]          # [50, TB], col = t*Bc + b
        out[c * Bc:(c + 1) * Bc] = lg.reshape(TAGS, T, Bc).transpose(2, 1, 0)
    return out


def bench(inputs):
    """Run once with NTFF tracing; returns HW exec_time_ns (and stashes trace)."""
    kernel(**inputs)  # ensure program built/cached
    nc = _CACHE["nc"]
    in_maps = _CACHE["in_maps"]
    import tempfile
    tmpdir = tempfile.mkdtemp(prefix="bilstm_trace_")
    res = run_bass_kernel_spmd(nc, in_maps, list(range(NCORES)), trace=True,
                               tmpdir=tmpdir)
    _CACHE["trace_dir"] = tmpdir
    _CACHE["last_bench"] = res
    print("trace dir:", tmpdir)
    if res.per_core_scope_times:
        for scope, times in res.per_core_scope_times.items():
            print(f"scope {scope}: {times}")
    return res.exec_time_ns


if __name__ == "__main__":
    import reference
    inputs = {k: np.asarray(v) for k, v in reference.setup_inputs().items()}
    got = kernel(**inputs)
    print(got.shape, got.dtype)
